# revision 7
# baseline (speedup 1.0000x reference)
"""Trainium (trn2) kernel for CurvedRoIExtractor (nn_CurvedRoIExtractor_28295244546862).

kernel(**inputs) takes the FULL inputs (as produced by setup_inputs()) and
returns the FULL output [2, 256, 256, 3, 16] f32.

Sharding: 8 cores = (batch b in {0,1}) x (64-roi quarter).  Features for the
core's batch are passed pre-transposed (channel-last, levels concatenated) so
the device can fetch the 4 bilinear-neighbor pixel rows of every sample point
with nc.gpsimd.dma_gather (1 KB contiguous per gathered pixel).  The weighted
sum over (level, neighbor) runs on TensorE via masked block-diagonal
matmuls accumulated in PSUM.  See the builder docstring below for layout
details.
"""

from contextlib import ExitStack

import numpy as np

import concourse.bass as bass
import concourse.mybir as mybir
import concourse.tile as tile
from concourse import library_config
from concourse.bass_utils import run_bass_kernel_spmd
from concourse.tile import add_dep_helper

F32 = mybir.dt.float32
F32R = mybir.dt.float32r
I16 = mybir.dt.int16
AOP = mybir.AluOpType

# (W, H, base row) of each feature level inside the concatenated table
LEVELS = [
    (160, 160, 0),
    (80, 80, 25600),
    (40, 40, 32000),
    (20, 20, 33600),
]
ROWS = 34048          # 34000 + padding rows (only weight-0 neighbors land there)
C = 256               # channels
BS = 2
NROI_TOTAL = 256
WP = 16
OUT_H = 3
NPTS = 3072           # per core: 64 rois * 3 * 16
NG_CHUNK = 16         # 32-point groups per chunk
MM_DTYPE = F32


def _fix_waits(nc, max_waits=1):
    """The walrus build in this env rejects >1 sem wait per instruction;
    spill extras onto preceding NOPs on the same engine."""
    for func in nc.m.functions:
        for bb in func.blocks:
            insts = bb.instructions
            for ins in list(insts):
                si = ins.sync_info
                if si is None:
                    continue
                w = list(si.on_wait)
                if len(w) > max_waits:
                    si.on_wait = w[:max_waits]
                    pos = insts.index(ins)
                    extra = w[max_waits:]
                    for k in range(0, len(extra), max_waits):
                        nop = mybir.InstNoOp(
                            name=f"{ins.name}-wf{k}",
                            engine=ins.engine,
                            bass_nofuse=True,
                            sync_info=mybir.SyncInfo(
                                on_wait=extra[k : k + max_waits], on_update=[]
                            ),
                        )
                        insts.insert(pos, nop)
                        pos += 1


def _build_kernel(levels=None, rows=ROWS, npts=NPTS, ng_chunk=NG_CHUNK,
                  mm_dtype=MM_DTYPE, fix=True):
    """Per-core program.

    Point order p = h*1024 + w*64 + roi' ; j = p%32, group g = p//32.
    Gather token order per level: t = 128*g + 32*n + j (n = bilinear
    neighbor 00,01,10,11) -> token t lands at partition t%128, block t//128,
    so group g's 128 (neighbor, point) rows fill all 128 partitions of
    block g.  Weighted sum over (level, neighbor): per (group, level) a
    matmul with masked block-diagonal lhsT[q, j'] = (q%32==j') *
    w_{l,q//32}[32g + q%32], accumulated over levels into PSUM [32, 256].
    """
    if levels is None:
        levels = LEVELS
    nlvl = len(levels)
    ngrp = npts // 32
    assert npts % 32 == 0 and ngrp % ng_chunk == 0
    nchunk = ngrp // ng_chunk
    icols = ngrp * 8          # idx table cols (= 4*npts/16)
    ccols = ng_chunk * 8      # idx cols per chunk

    nc = bass.Bass("TRN2", target_bir_lowering=False, num_devices=8)
    tf = nc.dram_tensor("tfeats", [rows, C], F32, kind="ExternalInput")
    gxd = nc.dram_tensor("gx", [32, ngrp], F32, kind="ExternalInput")
    gyd = nc.dram_tensor("gy", [32, ngrp], F32, kind="ExternalInput")
    maskd = nc.dram_tensor("mask", [128, 32], F32, kind="ExternalInput")
    outd = nc.dram_tensor("out", [npts, C], F32, kind="ExternalOutput")

    with tile.TileContext(nc) as tc, ExitStack() as ctx:
        prep = ctx.enter_context(tc.tile_pool(name="prep", bufs=1))
        gpool = ctx.enter_context(tc.tile_pool(name="g", bufs=2))
        lpool = ctx.enter_context(tc.tile_pool(name="lhs", bufs=2))
        spool = ctx.enter_context(tc.tile_pool(name="stage", bufs=3))
        ppool = ctx.enter_context(tc.tile_pool(name="ps", bufs=4, space="PSUM"))

        nc.gpsimd.load_library(library_config.attnmlp)

        gx = prep.tile([32, ngrp], F32, tag="gx")
        gy = prep.tile([32, ngrp], F32, tag="gy")
        mask = prep.tile([128, 32], F32, tag="mask")
        nc.sync.dma_start(gx[:], gxd[:])
        nc.sync.dma_start(gy[:], gyd[:])
        nc.sync.dma_start(mask[:], maskd[:])

        wcol = []   # per level [128, ngrp] weights (partition q = 32n+j)
        idxr = []   # per level [128, icols] int16 idx tables (replicated)
        for l, (W, H, base) in enumerate(levels):
            x = prep.tile([32, ngrp], F32, tag="x")
            y = prep.tile([32, ngrp], F32, tag="y")
            # match the reference's rounding: ((g + 1) * 0.5) * (W - 1)
            nc.vector.tensor_scalar(x[:], gx[:], 1.0, 0.5, AOP.add, AOP.mult)
            nc.vector.tensor_scalar(x[:], x[:], float(W - 1), None, AOP.mult)
            nc.vector.tensor_scalar(y[:], gy[:], 1.0, 0.5, AOP.add, AOP.mult)
            nc.vector.tensor_scalar(y[:], y[:], float(H - 1), None, AOP.mult)
            # floor(v) = round(v) - (round(v) > v), round via +/- 2^23
            # (exact: v in [0, 2^15), fp32 RN)
            wx = prep.tile([32, ngrp], F32, tag="wx")
            wy = prep.tile([32, ngrp], F32, tag="wy")
            x0 = prep.tile([32, ngrp], F32, tag="x0")
            y0 = prep.tile([32, ngrp], F32, tag="y0")
            M23 = 8388608.0
            for v, v0, frac in ((x, x0, wx), (y, y0, wy)):
                nc.vector.tensor_scalar(v0[:], v[:], M23, -M23, AOP.add, AOP.add)
                nc.vector.tensor_tensor(frac[:], v0[:], v[:], AOP.is_gt)
                nc.vector.tensor_tensor(v0[:], v0[:], frac[:], AOP.subtract)
                nc.vector.tensor_tensor(frac[:], v[:], v0[:], AOP.subtract)
            mx = prep.tile([32, ngrp], F32, tag="mx")  # 1-wx
            my = prep.tile([32, ngrp], F32, tag="my")  # 1-wy
            nc.vector.tensor_scalar(mx[:], wx[:], -1.0, 1.0, AOP.mult, AOP.add)
            nc.vector.tensor_scalar(my[:], wy[:], -1.0, 1.0, AOP.mult, AOP.add)

            wc = prep.tile([128, ngrp], F32, tag=f"wcol{l}")
            nc.vector.tensor_tensor(wc[0:32, :], mx[:], my[:], AOP.mult)
            nc.vector.tensor_tensor(wc[32:64, :], wx[:], my[:], AOP.mult)
            nc.vector.tensor_tensor(wc[64:96, :], mx[:], wy[:], AOP.mult)
            nc.vector.tensor_tensor(wc[96:128, :], wx[:], wy[:], AOP.mult)
            wcol.append(wc)

            # idx00 = y0*W + x0; neighbor offsets {0, 1, W, W+1}
            i00 = prep.tile([32, ngrp], F32, tag="i00")
            nc.vector.tensor_scalar(i00[:], y0[:], float(W), None, AOP.mult)
            nc.vector.tensor_tensor(i00[:], i00[:], x0[:], AOP.add)
            iall = prep.tile([32, 4 * ngrp], I16, tag=f"iall{l}")
            for n, off in enumerate([0.0, 1.0, float(W), float(W + 1)]):
                nc.vector.tensor_scalar(
                    iall[:, n * ngrp : (n + 1) * ngrp],
                    i00[:], off, None, AOP.add,
                )

            # wrapped idx table [16, icols]: [r, 8g+2n+u] = iall[16u+r, n*G+g]
            i16 = prep.tile([16, icols], I16, tag=f"i16_{l}")
            dview = i16[:].rearrange("p (g n u) -> p n g u", n=4, u=2)
            for u in range(2):
                for n in range(4):
                    nc.sync.dma_start(
                        out=dview[:, n, :, u],
                        in_=iall[16 * u : 16 * u + 16,
                                 n * ngrp : (n + 1) * ngrp],
                    )
            rep = prep.tile([128, icols], I16, tag=f"irep{l}")
            for k in range(8):
                nc.sync.dma_start(rep[16 * k : 16 * k + 16, :], i16[:])
            idxr.append(rep)

        mask_b = mask[:].unsqueeze(1).to_broadcast([128, ng_chunk, 32])

        for ch in range(nchunk):
            gts = []
            for l, (W, H, base) in enumerate(levels):
                gt = gpool.tile([128, ng_chunk, C], F32, tag=f"g{l}")
                hi = min(base + W * H + 2 * W + 2, rows)
                nc.gpsimd.dma_gather(
                    out_ap=gt[:, :, :],
                    in_ap=tf[base:hi, :],
                    idxs_ap=idxr[l][:, ch * ccols : (ch + 1) * ccols],
                    num_idxs=ng_chunk * 128,
                    num_idxs_reg=ng_chunk * 128,
                    elem_size=C,
                    single_packet=False,
                )
                gts.append(gt)
            lhs = []
            for l in range(nlvl):
                lt = lpool.tile([128, ng_chunk * 32], F32, tag=f"w{l}")
                wslice = wcol[l][:, ch * ng_chunk : (ch + 1) * ng_chunk]
                nc.vector.tensor_tensor(
                    lt[:].rearrange("p (g k) -> p g k", k=32),
                    mask_b,
                    wslice.to_broadcast([128, ng_chunk, 32]),
                    AOP.mult,
                )
                lhs.append(lt)

            prev_mm = None
            for cl in range(ng_chunk // 4):
                ps = ppool.tile([128, C], F32, tag="ps")
                for a in range(4):
                    gi = cl * 4 + a
                    for l in range(nlvl):
                        mm = nc.tensor.matmul(
                            ps[32 * a : 32 * a + 32, :],
                            lhs[l][:, 32 * gi : 32 * (gi + 1)].bitcast(mm_dtype),
                            gts[l][:, gi, :].bitcast(mm_dtype),
                            start=(l == 0),
                            stop=(l == nlvl - 1),
                            tile_position=(0, 32 * a),
                        )
                        # Force PE order: accumulation chains sharing a PSUM
                        # bank must not interleave (start=True clears the
                        # whole bank's has_written bits).
                        if prev_mm is not None:
                            add_dep_helper(mm.ins, prev_mm.ins, sync=False)
                        prev_mm = mm
                st = spool.tile([128, C], F32, tag="st")
                nc.vector.tensor_copy(out=st[:], in_=ps[:])
                row0 = ch * ng_chunk * 32 + cl * 128
                nc.sync.dma_start(outd[row0 : row0 + 128, :], st[:])

    mybir.codegen_inst_isa_subclasses(nc)
    if fix:
        _fix_waits(nc)
    return nc


# ---------------------------------------------------------------------------
# Host-side prep

def _host_prep_points(center_b, boundary_b, roi0, nroi):
    bp = boundary_b[roi0 : roi0 + nroi]      # [nroi, Wp, 4]
    cp = center_b[roi0 : roi0 + nroi]        # [nroi, Wp, 2]
    sp = np.stack([bp[..., 0:2], cp, bp[..., 2:4]], axis=1)  # [nroi,3,Wp,2]
    gxa = np.ascontiguousarray(sp[..., 0].transpose(1, 2, 0)).reshape(-1)
    gya = np.ascontiguousarray(sp[..., 1].transpose(1, 2, 0)).reshape(-1)
    npts = gxa.size
    gx = gxa.reshape(npts // 32, 32).T.copy()
    gy = gya.reshape(npts // 32, 32).T.copy()
    return gx.astype(np.float32), gy.astype(np.float32)


def _host_mask():
    q = np.arange(128)[:, None] % 32
    j = np.arange(32)[None, :]
    return (q == j).astype(np.float32)


def _host_tfeats(feats_b_list, rows=ROWS):
    parts = [np.ascontiguousarray(f.reshape(f.shape[0], -1).T)
             for f in feats_b_list]
    tfx = np.concatenate(parts, axis=0)
    pad = rows - tfx.shape[0]
    if pad:
        tfx = np.concatenate(
            [tfx, np.zeros((pad, tfx.shape[1]), np.float32)], axis=0)
    return np.ascontiguousarray(tfx.astype(np.float32))


_CACHE = {}


def _get_nc():
    if "nc" not in _CACHE:
        _CACHE["nc"] = _build_kernel()
    return _CACHE["nc"]


def kernel(feats0, feats1, feats2, feats3, center_points, boundary_points,
           _want_trace=False, _trace_dir=None):
    feats0 = np.asarray(feats0, dtype=np.float32)
    feats1 = np.asarray(feats1, dtype=np.float32)
    feats2 = np.asarray(feats2, dtype=np.float32)
    feats3 = np.asarray(feats3, dtype=np.float32)
    center_points = np.asarray(center_points, dtype=np.float32)
    boundary_points = np.asarray(boundary_points, dtype=np.float32)

    nc = _get_nc()
    mask = _host_mask()
    tfeats = [
        _host_tfeats([feats0[b], feats1[b], feats2[b], feats3[b]])
        for b in range(BS)
    ]
    nroi = NROI_TOTAL // 4  # 64 rois per core
    in_maps = []
    for core in range(8):
        b = core // 4
        roi0 = (core % 4) * nroi
        gx, gy = _host_prep_points(
            center_points[b], boundary_points[b], roi0, nroi)
        in_maps.append(
            {"tfeats": tfeats[b], "gx": gx, "gy": gy, "mask": mask})

    kwargs = {}
    if _want_trace:
        kwargs = {"trace": True}
        if _trace_dir is not None:
            kwargs["tmpdir"] = _trace_dir
    res = run_bass_kernel_spmd(nc, in_maps, core_ids=list(range(8)), **kwargs)

    out = np.empty((BS, NROI_TOTAL, C, OUT_H, WP), np.float32)
    for core in range(8):
        b = core // 4
        roi0 = (core % 4) * nroi
        dev = res.results[core]["out"]          # [NPTS, C], rows (h, w, roi')
        o = dev.reshape(OUT_H, WP, nroi, C).transpose(2, 3, 0, 1)
        out[b, roi0 : roi0 + nroi] = o
    if _want_trace:
        return out, res
    return out
